# revision 30
# baseline (speedup 1.0000x reference)
"""Trainium2 Bass kernel for nn_Blur (upfirdn2d 4x4 blur, pad=(2,1)).

Formulation: out[i,j] = sum_{p,q} Kf[p,q] * x[i+p-2, j+q-2]   (Kf = flip(kernel2d))

For each W-tap q (4 taps), the H-convolution is a banded 64x64 matrix
Aq[i,h] = Kf[h-i+2, q].  Tolerance is 2e-2, so x streams as a single bf16
(the {1,3,9}/64 blur weights have <=4 mantissa bits: every bf16 product is
exact in fp32; end-to-end error ~5e-3) -- HALF the HBM traffic of an
fp32-faithful hi/lo split.

The K=128 contraction is filled by stacking TWO images per partition set:
lhsT_q = blockdiag(Aq^T, Aq^T) [128,128], rhs = [x_even; x_odd] [128, N],
so each matmul computes both images' H-conv at M=128 (full PE width, no
tile_position games).  The 4 taps accumulate into one PSUM bank with
variable-width windows: tap q=2 covers the full width first (start=True
initializes the per-element has_written state everywhere), the narrower
boundary taps then accumulate into column subsets.  This keeps the PE
~90% busy, which holds the clock-boost (HAM) state -- schemes with less
tensor work demote the PE clock to 1.2 GHz and end up slower.

The fp32 PSUM result is copied to SBUF as bf16 (alternating vector /
scalar engines), DMA'd back as [128,512] bf16 tiles, and cast to f32 on
the host.  HBM per core: 8.4 MB in + 8.4 MB out = the ~47us roofline.

Sharding: the 16*512 = 8192 independent (n,c) images are split into 8
contiguous slabs of 1024 images, one per NeuronCore (data-parallel).
"""

import ml_dtypes
import numpy as np

import concourse.bacc as bacc
import concourse.bass as bass
import concourse.mybir as mybir
import concourse.tile as tile
from concourse.bass_utils import run_bass_kernel_spmd

N_CORES = 8
IMG = 64                      # H = W
N_IMAGES = 16 * 512           # 8192
PER_CORE = N_IMAGES // N_CORES  # 1024
GROUP = 16                    # images per batch (8 pairs stacked in K)
N_BATCH = PER_CORE // GROUP   # 64
N_SUPER = N_BATCH // 4        # 16 DMA supertiles (4 batches each)
N_BD_SUPER = 4                # supers 0-3: block-diag (fill armor)
N_PAIR_SUPER = N_SUPER - N_BD_SUPER  # supers 4-15: pair-fused taps
TILE_W = 8 * IMG              # 512 free cols per batch: 8 image pairs
# per-tap W windows: tap q reads x cols [XLO[q], +LEN[q]) and writes out
# cols [JLO[q], +LEN[q]).  q=2 goes first: full width, start=True.
TAP_ORDER = (2, 0, 1, 3)
XLO = (0, 0, 0, 1)
JLO = (2, 1, 0, 0)
LEN = (62, 63, 64, 63)
DT = mybir.dt.float32
IN_DT = mybir.dt.bfloat16
NP_IN = ml_dtypes.bfloat16

LAST_RESULTS = None  # BassKernelResults of the most recent run (for test.py)


def _build_weights(kernel2d: np.ndarray) -> np.ndarray:
    """[128, 512] bf16: cols [128q, 128q+128) = blockdiag(Aq^T, Aq^T)."""
    kf = np.flip(np.asarray(kernel2d, dtype=np.float64), (0, 1))
    wts = np.zeros((128, 512), dtype=NP_IN)
    for q in range(4):
        aq = np.zeros((IMG, IMG), dtype=np.float64)
        for i in range(IMG):
            for p in range(4):
                h = i + p - 2
                if 0 <= h < IMG:
                    aq[i, h] = kf[p, q]
        aqt = aq.T.astype(NP_IN)
        wts[:IMG, 128 * q : 128 * q + IMG] = aqt
        wts[IMG:, 128 * q + IMG : 128 * q + 128] = aqt
    return wts


def _build_pair_weights(kernel2d: np.ndarray) -> np.ndarray:
    """[128, 256] bf16: cols 0:64=[A2^T;A3^T], 64:128=[A0^T;A1^T],
    128:192=[A2^T;0] (tap2 boundary), 192:256=[A1^T;0] (tap1 boundary)."""
    kf = np.flip(np.asarray(kernel2d, dtype=np.float64), (0, 1))
    a = np.zeros((4, IMG, IMG), dtype=np.float64)
    for q in range(4):
        for i in range(IMG):
            for p in range(4):
                h = i + p - 2
                if 0 <= h < IMG:
                    a[q, i, h] = kf[p, q]
    wts = np.zeros((128, 256), dtype=NP_IN)
    wts[:IMG, 0:IMG] = a[2].T.astype(NP_IN)
    wts[IMG:, 0:IMG] = a[3].T.astype(NP_IN)
    wts[:IMG, IMG:128] = a[0].T.astype(NP_IN)
    wts[IMG:, IMG:128] = a[1].T.astype(NP_IN)
    wts[:IMG, 128:192] = a[2].T.astype(NP_IN)
    wts[:IMG, 192:256] = a[1].T.astype(NP_IN)
    return wts


def _bass_module() -> bass.Bass:
    nc = bacc.Bacc(
        "TRN2",
        target_bir_lowering=False,
        debug=False,
        num_devices=N_CORES,
    )
    x_d = nc.dram_tensor(
        "x", [N_BD_SUPER, 128, 4 * TILE_W], IN_DT, kind="ExternalInput"
    )
    xp_d = nc.dram_tensor(
        "xp", [N_PAIR_SUPER, IMG, 4 * 2 * TILE_W], IN_DT, kind="ExternalInput"
    )
    w_d = nc.dram_tensor("wts", [128, 512], IN_DT, kind="ExternalInput")
    w2_d = nc.dram_tensor("wts2", [128, 256], IN_DT, kind="ExternalInput")
    o_d = nc.dram_tensor(
        "out", [N_SUPER, 128, 4 * 512], IN_DT, kind="ExternalOutput"
    )

    with tile.TileContext(nc) as tc:
        with (
            tc.tile_pool(name="const", bufs=1) as cpool,
            tc.tile_pool(name="inp", bufs=5) as ipool,
            tc.tile_pool(name="pinp", bufs=5) as ppinp,
            tc.tile_pool(name="outp", bufs=3) as opool,
            tc.tile_pool(name="psum", bufs=8, space="PSUM") as ppool,
        ):
            w_tile = cpool.tile([128, 512], IN_DT)
            # first weight block lands first (32KB) so warmup can start
            # ~2us earlier; the rest follows while warmup runs.
            nc.sync.dma_start(w_tile[:, 0:256], w_d[:, 0:256])
            nc.sync.dma_start(w_tile[:, 256:512], w_d[:, 256:512])
            w2_tile = cpool.tile([128, 256], IN_DT)
            nc.sync.dma_start(w2_tile[:], w2_d[:])

            # HAM warmup: the PE clock-gate holds 1.2 GHz until ~3.4us of
            # sustained matmul activity.  Burn that window on dummy matmuls
            # reading the (just-DMA'd) first weight block as both operands.
            warm_ps = ppool.tile([128, 512], DT, tag="ps")
            for _ in range(16):
                nc.tensor.matmul(
                    warm_ps[:, 0:256],
                    w_tile[:, 0:128],
                    w_tile[:, 0:256],
                    start=True,
                    stop=True,
                )

            # DMA granularity: every dma_start costs ~600ns of descriptor
            # generation on its issuing sequencer (software DGE), so DMAs
            # move 4 batches at a time (supertiles); compute, PSUM and
            # copies stay at single-batch granularity.
            LOOKAHEAD = 4
            PW = 2 * TILE_W  # 1024 free cols per pair batch
            in_tiles = {}
            pair_tiles = {}

            def issue_in(s):
                t = ipool.tile([128, 4 * TILE_W], IN_DT)
                nc.sync.dma_start(t[:], x_d[s])
                in_tiles[s] = t

            def issue_pair_in(sp):
                t = ppinp.tile([128, 4 * PW], IN_DT)
                nc.sync.dma_start(t[0:IMG, :], xp_d[sp])
                pair_tiles[sp] = t

            def issue_dup(sp):
                # shifted dup: partitions 64-127 = x rows one col left
                # (sync queue path; spread across the 16 HW DMA queues so
                # the burst never stalls PE rhs fetches).  Issued 2 supers
                # after its input DMA so the wait is already satisfied.
                t = pair_tiles[sp]
                nc.sync.dma_start(
                    t[IMG:128, 0 : 4 * PW - 1], t[0:IMG, 1 : 4 * PW]
                )

            for s in range(N_BD_SUPER):
                issue_in(s)

            out_super = None
            for b in range(N_BATCH):
                s, k = divmod(b, 4)
                if k == 0:
                    if s < N_PAIR_SUPER:
                        issue_pair_in(s)
                    if 2 <= s and s - 2 < N_PAIR_SUPER:
                        issue_dup(s - 2)
                    if s < N_BD_SUPER:
                        in_super = in_tiles.pop(s)
                    else:
                        in_super = pair_tiles.pop(s - N_BD_SUPER)
                    out_super = opool.tile([128, 4 * 512], IN_DT)

                ps = ppool.tile([128, 512], DT)
                if s < N_BD_SUPER:
                    rhs3 = in_super[
                        :, k * TILE_W : (k + 1) * TILE_W
                    ].rearrange("p (g w) -> p g w", w=IMG)
                    out3 = ps[:].rearrange("p (g w) -> p g w", w=IMG)
                    for qi, q in enumerate(TAP_ORDER):
                        nc.tensor.matmul(
                            out3[:, :, JLO[q] : JLO[q] + LEN[q]],
                            w_tile[:, 128 * q : 128 * q + 128],
                            rhs3[:, :, XLO[q] : XLO[q] + LEN[q]],
                            start=(qi == 0),
                            stop=(qi == 3),
                        )
                else:
                    rhs3 = in_super[:, k * PW : (k + 1) * PW].rearrange(
                        "p (g w) -> p g w", w=IMG
                    )
                    out3s = [
                        ps[cg * IMG : (cg + 1) * IMG, :].rearrange(
                            "p (g w) -> p g w", w=IMG
                        )
                        for cg in range(2)
                    ]
                    gss = [slice(cg * 8, (cg + 1) * 8) for cg in range(2)]
                    # tap2@j=63 first: its start=True initializes the
                    # per-element has_written state per column group
                    for cg in range(2):
                        nc.tensor.matmul(
                            out3s[cg][:, :, IMG - 1 : IMG],
                            w2_tile[0:IMG, 128:192],
                            rhs3[0:IMG, gss[cg], IMG - 1 : IMG],
                            start=True,
                            stop=False,
                            tile_position=(0, cg * IMG),
                            skip_group_check=True,
                        )
                    # pair(2,3): stores into cleared elements j=0..62
                    for cg in range(2):
                        nc.tensor.matmul(
                            out3s[cg][:, :, 0 : IMG - 1],
                            w2_tile[:, 0:IMG],
                            rhs3[:, gss[cg], 0 : IMG - 1],
                            start=False,
                            stop=False,
                            tile_position=(0, cg * IMG),
                            skip_group_check=True,
                        )
                    # tap1@j=1 accumulates
                    for cg in range(2):
                        nc.tensor.matmul(
                            out3s[cg][:, :, 1:2],
                            w2_tile[0:IMG, 192:256],
                            rhs3[0:IMG, gss[cg], 0:1],
                            start=False,
                            stop=False,
                            tile_position=(0, cg * IMG),
                            skip_group_check=True,
                        )
                    # pair(0,1): accumulates into j=2..63
                    for cg in range(2):
                        nc.tensor.matmul(
                            out3s[cg][:, :, 2:IMG],
                            w2_tile[:, IMG:128],
                            rhs3[:, gss[cg], 0 : IMG - 2],
                            start=False,
                            stop=True,
                            tile_position=(0, cg * IMG),
                            skip_group_check=True,
                        )

                dst = out_super[:, k * 512 : (k + 1) * 512]
                if b % 2 == 0:
                    nc.vector.tensor_copy(dst, ps[:])
                else:
                    nc.scalar.copy(dst, ps[:])
                if s == N_SUPER - 1:
                    # final supertile: per-batch out-DMAs overlap the last
                    # copies instead of waiting for all four
                    eng = nc.sync if k % 2 == 0 else nc.scalar
                    eng.dma_start(
                        o_d[s][:, k * 512 : (k + 1) * 512], dst
                    )
                elif k == 3:
                    # alternate the issuing sequencer per supertile
                    eng = nc.sync if s % 2 == 0 else nc.scalar
                    eng.dma_start(o_d[s], out_super[:])
    nc.compile()
    return nc


def _host_pack(x: np.ndarray):
    """FULL x (8192,64,64) f32 -> (bd, pair) bf16 shards.

    bd   [N_CORES, N_BD_SUPER, 128, 2048]: partition (a, h), a = image
         parity in pair; free (sub, g: 8 pairs, w); img = b*16 + g*2 + a.
    pair [N_CORES, N_PAIR_SUPER, 64, 4096]: partition h; free (sub,
         g: 16 images, w); img = b*16 + g."""
    xb = x.reshape(N_CORES, N_BATCH, GROUP, IMG, IMG)
    nbd = N_BD_SUPER * 4
    v = xb[:, :nbd].reshape(N_CORES, nbd, 8, 2, IMG, IMG).transpose(
        0, 1, 3, 4, 2, 5
    )
    flat = np.ascontiguousarray(v.astype(NP_IN)).reshape(
        N_CORES, N_BD_SUPER, 4, 128, TILE_W
    )
    bd = np.ascontiguousarray(flat.transpose(0, 1, 3, 2, 4)).reshape(
        N_CORES, N_BD_SUPER, 128, 4 * TILE_W
    )
    v2 = xb[:, nbd:].transpose(0, 1, 3, 2, 4)  # [core, b, h, g, w]
    flat2 = np.ascontiguousarray(v2.astype(NP_IN)).reshape(
        N_CORES, N_PAIR_SUPER, 4, IMG, 2 * TILE_W
    )
    pair = np.ascontiguousarray(flat2.transpose(0, 1, 3, 2, 4)).reshape(
        N_CORES, N_PAIR_SUPER, IMG, 4 * 2 * TILE_W
    )
    return bd, pair


def _host_unpack(tiles: np.ndarray) -> np.ndarray:
    """[N_CORES, N_SUPER, 128, 2048] bf16 -> (8192, 64, 64) f32."""
    v = tiles.reshape(N_CORES, N_SUPER, 128, 4, 512)
    v = v.transpose(0, 1, 3, 2, 4).reshape(N_CORES, N_BATCH, 2, IMG, 8, IMG)
    nbd = N_BD_SUPER * 4
    out = np.empty((N_CORES, N_BATCH, GROUP, IMG, IMG), dtype=v.dtype)
    # bd batches: partitions (a, i), free (g, j); img = g*2 + a
    out[:, :nbd] = (
        v[:, :nbd]
        .transpose(0, 1, 4, 2, 3, 5)
        .reshape(N_CORES, nbd, GROUP, IMG, IMG)
    )
    # pair batches: partitions (cg, i), free (g, j); img = cg*8 + g
    out[:, nbd:] = (
        v[:, nbd:]
        .transpose(0, 1, 2, 4, 3, 5)
        .reshape(N_CORES, N_BATCH - nbd, GROUP, IMG, IMG)
    )
    return out.reshape(N_IMAGES, IMG, IMG).astype(np.float32)


def kernel(x: np.ndarray, kernel: np.ndarray, _trace: bool = False) -> np.ndarray:
    global LAST_RESULTS
    x = np.ascontiguousarray(np.asarray(x, dtype=np.float32))
    n, c, h, w = x.shape
    assert (n, c, h, w) == (16, 512, 64, 64), x.shape

    bd, pair = _host_pack(x.reshape(N_IMAGES, IMG, IMG))
    wts = _build_weights(kernel)
    wts2 = _build_pair_weights(kernel)
    in_maps = [
        {"x": bd[i], "xp": pair[i], "wts": wts, "wts2": wts2}
        for i in range(N_CORES)
    ]

    nc = _bass_module()
    results = run_bass_kernel_spmd(
        nc, in_maps, core_ids=list(range(N_CORES)), trace=_trace
    )
    LAST_RESULTS = results

    tiles = np.stack([r["out"] for r in results.results])
    out = _host_unpack(tiles)
    return np.ascontiguousarray(out.reshape(n, c, h, w)).astype(np.float32)


# revision 31
# speedup vs baseline: 1.5373x; 1.5373x over previous
"""Trainium2 Bass kernel for nn_Blur (upfirdn2d 4x4 blur, pad=(2,1)).

Formulation: out[i,j] = sum_{p,q} Kf[p,q] * x[i+p-2, j+q-2]   (Kf = flip(kernel2d))

For each W-tap q (4 taps), the H-convolution is a banded 64x64 matrix
Aq[i,h] = Kf[h-i+2, q].  Tolerance is 2e-2, so x streams as a single bf16
(the {1,3,9}/64 blur weights have <=4 mantissa bits: every bf16 product is
exact in fp32; end-to-end error ~5e-3) -- HALF the HBM traffic of an
fp32-faithful hi/lo split.

The K=128 contraction is filled by stacking TWO images per partition set:
lhsT_q = blockdiag(Aq^T, Aq^T) [128,128], rhs = [x_even; x_odd] [128, N],
so each matmul computes both images' H-conv at M=128 (full PE width, no
tile_position games).  The 4 taps accumulate into one PSUM bank with
variable-width windows: tap q=2 covers the full width first (start=True
initializes the per-element has_written state everywhere), the narrower
boundary taps then accumulate into column subsets.  This keeps the PE
~90% busy, which holds the clock-boost (HAM) state -- schemes with less
tensor work demote the PE clock to 1.2 GHz and end up slower.

The fp32 PSUM result is copied to SBUF as bf16 (alternating vector /
scalar engines), DMA'd back as [128,512] bf16 tiles, and cast to f32 on
the host.  HBM per core: 8.4 MB in + 8.4 MB out = the ~47us roofline.

Sharding: the 16*512 = 8192 independent (n,c) images are split into 8
contiguous slabs of 1024 images, one per NeuronCore (data-parallel).
"""

import ml_dtypes
import numpy as np

import concourse.bacc as bacc
import concourse.bass as bass
import concourse.mybir as mybir
import concourse.tile as tile
from concourse.bass_utils import run_bass_kernel_spmd

N_CORES = 8
IMG = 64                      # H = W
N_IMAGES = 16 * 512           # 8192
PER_CORE = N_IMAGES // N_CORES  # 1024
GROUP = 16                    # images per batch (8 pairs stacked in K)
N_BATCH = PER_CORE // GROUP   # 64
N_SUPER = N_BATCH // 4        # 16 DMA supertiles (4 batches each)
TILE_W = 8 * IMG              # 512 free cols per batch: 8 image pairs
# per-tap W windows: tap q reads x cols [XLO[q], +LEN[q]) and writes out
# cols [JLO[q], +LEN[q]).  q=2 goes first: full width, start=True.
TAP_ORDER = (2, 0, 1, 3)
XLO = (0, 0, 0, 1)
JLO = (2, 1, 0, 0)
LEN = (62, 63, 64, 63)
DT = mybir.dt.float32
IN_DT = mybir.dt.bfloat16
NP_IN = ml_dtypes.bfloat16

LAST_RESULTS = None  # BassKernelResults of the most recent run (for test.py)


def _build_weights(kernel2d: np.ndarray) -> np.ndarray:
    """[128, 512] bf16: cols [128q, 128q+128) = blockdiag(Aq^T, Aq^T)."""
    kf = np.flip(np.asarray(kernel2d, dtype=np.float64), (0, 1))
    wts = np.zeros((128, 512), dtype=NP_IN)
    for q in range(4):
        aq = np.zeros((IMG, IMG), dtype=np.float64)
        for i in range(IMG):
            for p in range(4):
                h = i + p - 2
                if 0 <= h < IMG:
                    aq[i, h] = kf[p, q]
        aqt = aq.T.astype(NP_IN)
        wts[:IMG, 128 * q : 128 * q + IMG] = aqt
        wts[IMG:, 128 * q + IMG : 128 * q + 128] = aqt
    return wts


def _bass_module() -> bass.Bass:
    nc = bacc.Bacc(
        "TRN2",
        target_bir_lowering=False,
        debug=False,
        num_devices=N_CORES,
    )
    x_d = nc.dram_tensor(
        "x", [N_SUPER, 128, 4 * TILE_W], IN_DT, kind="ExternalInput"
    )
    w_d = nc.dram_tensor("wts", [128, 512], IN_DT, kind="ExternalInput")
    o_d = nc.dram_tensor(
        "out", [N_SUPER, 128, 4 * 512], IN_DT, kind="ExternalOutput"
    )

    with tile.TileContext(nc) as tc:
        with (
            tc.tile_pool(name="const", bufs=1) as cpool,
            tc.tile_pool(name="inp", bufs=5) as ipool,
            tc.tile_pool(name="outp", bufs=3) as opool,
            tc.tile_pool(name="psum", bufs=8, space="PSUM") as ppool,
        ):
            w_tile = cpool.tile([128, 512], IN_DT)
            # first weight block lands first (32KB) so warmup can start
            # ~2us earlier; the rest follows while warmup runs.
            nc.sync.dma_start(w_tile[:, 0:256], w_d[:, 0:256])
            nc.sync.dma_start(w_tile[:, 256:512], w_d[:, 256:512])

            # HAM warmup: the PE clock-gate holds 1.2 GHz until ~3.4us of
            # sustained matmul activity.  Burn that window on dummy matmuls
            # reading the (just-DMA'd) first weight block as both operands.
            warm_ps = ppool.tile([128, 512], DT, tag="ps")
            for _ in range(16):
                nc.tensor.matmul(
                    warm_ps[:, 0:256],
                    w_tile[:, 0:128],
                    w_tile[:, 0:256],
                    start=True,
                    stop=True,
                )

            # DMA granularity: every dma_start costs ~600ns of descriptor
            # generation on its issuing sequencer (software DGE), so DMAs
            # move 4 batches at a time (supertiles); compute, PSUM and
            # copies stay at single-batch granularity.
            LOOKAHEAD = 4
            in_tiles = {}

            def issue_in(s):
                t = ipool.tile([128, 4 * TILE_W], IN_DT)
                nc.sync.dma_start(t[:], x_d[s])
                in_tiles[s] = t

            for s in range(LOOKAHEAD):
                issue_in(s)

            out_super = None
            for b in range(N_BATCH):
                s, k = divmod(b, 4)
                if k == 0:
                    if s + LOOKAHEAD < N_SUPER:
                        issue_in(s + LOOKAHEAD)
                    in_super = in_tiles.pop(s)
                    out_super = opool.tile([128, 4 * 512], IN_DT)
                rhs3 = in_super[:, k * TILE_W : (k + 1) * TILE_W].rearrange(
                    "p (g w) -> p g w", w=IMG
                )

                ps = ppool.tile([128, 512], DT)
                out3 = ps[:].rearrange("p (g w) -> p g w", w=IMG)
                for qi, q in enumerate(TAP_ORDER):
                    nc.tensor.matmul(
                        out3[:, :, JLO[q] : JLO[q] + LEN[q]],
                        w_tile[:, 128 * q : 128 * q + 128],
                        rhs3[:, :, XLO[q] : XLO[q] + LEN[q]],
                        start=(qi == 0),
                        stop=(qi == 3),
                    )

                dst = out_super[:, k * 512 : (k + 1) * 512]
                if b % 2 == 0:
                    nc.vector.tensor_copy(dst, ps[:])
                else:
                    nc.scalar.copy(dst, ps[:])
                if s == N_SUPER - 1:
                    # final supertile: per-batch out-DMAs overlap the last
                    # copies instead of waiting for all four
                    eng = nc.sync if k % 2 == 0 else nc.scalar
                    eng.dma_start(
                        o_d[s][:, k * 512 : (k + 1) * 512], dst
                    )
                elif k == 3:
                    # alternate the issuing sequencer per supertile
                    eng = nc.sync if s % 2 == 0 else nc.scalar
                    eng.dma_start(o_d[s], out_super[:])
    nc.compile()
    return nc


def _host_pack(x: np.ndarray) -> np.ndarray:
    """FULL x (8192,64,64) f32 -> [N_CORES, N_BATCH, 128, 512] bf16.

    Partition dim = (a, h), a = image parity in pair; free dim = (g: 8
    pairs, w).  img = core*1024 + b*16 + g*2 + a."""
    v = x.reshape(N_CORES, N_BATCH, 8, 2, IMG, IMG).transpose(0, 1, 3, 4, 2, 5)
    flat = np.ascontiguousarray(v.astype(NP_IN)).reshape(
        N_CORES, N_SUPER, 4, 128, TILE_W
    )
    return np.ascontiguousarray(flat.transpose(0, 1, 3, 2, 4)).reshape(
        N_CORES, N_SUPER, 128, 4 * TILE_W
    )


def _host_unpack(tiles: np.ndarray) -> np.ndarray:
    """[N_CORES, N_BATCH, 128, 512] bf16 -> (8192, 64, 64) f32."""
    v = tiles.reshape(N_CORES, N_SUPER, 128, 4, 512)
    v = v.transpose(0, 1, 3, 2, 4).reshape(N_CORES, N_BATCH, 2, IMG, 8, IMG)
    v = v.transpose(0, 1, 4, 2, 3, 5)  # [core, b, g, a, i, j]
    return v.reshape(N_IMAGES, IMG, IMG).astype(np.float32)


def kernel(x: np.ndarray, kernel: np.ndarray, _trace: bool = False) -> np.ndarray:
    global LAST_RESULTS
    x = np.ascontiguousarray(np.asarray(x, dtype=np.float32))
    n, c, h, w = x.shape
    assert (n, c, h, w) == (16, 512, 64, 64), x.shape

    shards = _host_pack(x.reshape(N_IMAGES, IMG, IMG))
    wts = _build_weights(kernel)
    in_maps = [{"x": shards[i], "wts": wts} for i in range(N_CORES)]

    nc = _bass_module()
    results = run_bass_kernel_spmd(
        nc, in_maps, core_ids=list(range(N_CORES)), trace=_trace
    )
    LAST_RESULTS = results

    tiles = np.stack([r["out"] for r in results.results])
    out = _host_unpack(tiles)
    return np.ascontiguousarray(out.reshape(n, c, h, w)).astype(np.float32)
